# revision 36
# baseline (speedup 1.0000x reference)
"""Trainium2 Bass kernel for nn_Decoder_fusion (sparse_attention).

Data-parallel over batch B=8 across 8 NeuronCores (one sample per core).
Per-core layout: channel-major [C=128 partitions, L=4096 tokens], fp16
activations (fp32 PSUM accumulation everywhere).

Two dwblocks run software-pipelined: each block is emitted by a
generator that yields between micro-phases, and a round-robin driver
interleaves two blocks' instruction streams so the in-order engines
overlap block n's vector/DMA phases with block n+1's PE phases.
All transient buffers are parity-tagged (blki % 2).

Per dwblock:
  pw conv   -> PE matmuls; PSUM->SBUF copy on ACT folds the conv bias
  LayerNorm -> per-token stats via PE ones-reduction matmuls (Square on
               DVE); stats rows staged by ACT, reshaped by DMA; rstd via
               DVE Newton rsqrt (no ACT table swaps); u/-mu*u rows
               broadcast to all partitions by DMA; apply is two fp16 2x
               DVE tensor_tensor passes; gamma/beta ride the ACT
               Gelu/Identity pass as per-partition scale/bias
  depthwise -> 27 taps: PE diag-matmuls into PSUM plus a per-quarter
               DVE/Pool fused mul-add chain into a dense accumulator
               merged through the PE
Attention: q/K transposed tile-wise on PE; per-K-block logits matmuls
accumulate into an SBUF fp32 tile; masked softmax on a [128,512] tile;
attn@V per modality accumulated into SBUF by DVE.
The fp32 residual (query + x) is added on the host.
"""

import os
import sys

sys.path.insert(0, "/opt/trn_rl_repo")

import contextlib

import numpy as np

import bass_rust
import concourse.bass as bass
import concourse.mybir as mybir
import concourse.tile as tile
from concourse.bass_utils import run_bass_kernel_spmd

# Old walrus encodes EVENT_SEMAPHORE_RANGE_CLEAR / drain-reset ranges of at
# most 9 semaphores; cap the ranges bass emits at tile-context exit.
_orig_ctr = bass.compact_to_ranges


def _capped_ctr(vals):
    out = []
    for r in _orig_ctr(vals):
        vs = list(r)
        for i in range(0, len(vs), 9):
            chunk = vs[i : i + 9]
            out.append(range(chunk[0], chunk[-1] + 1))
    return out


bass.compact_to_ranges = _capped_ctr

F32 = mybir.dt.float32
F16 = mybir.dt.float16
I32 = mybir.dt.int32
AF = mybir.ActivationFunctionType
OP = mybir.AluOpType
AX = mybir.AxisListType

KSIM = bool(int(os.environ.get("KSIM", "0")))
B, C, S = 8, 128, 16
L = S * S * S            # 4096
PX = S + 2               # 18
PL = PX * PX * PX        # 5832
NBLK = 10
NMOD = 4
NQ = 4                   # quarters per volume
QL = L // NQ             # 1024 tokens per quarter
EPS = 1e-6

# Depthwise tap split: PE diag-matmuls vs DVE/Pool fused mul-add chain.
# VEC_TAPS entries are (tap, engine) with engine "D" (DVE) or "P" (Pool).
VEC_TAPS = ((4, "D"), (13, "D"), (22, "D"), (10, "D"), (16, "D"))
PE_TAPS = tuple(t for t in range(27)
                if t not in tuple(v[0] for v in VEC_TAPS))
NPE = len(PE_TAPS)


def _mm(nc, out, lhsT, rhs, start=True, stop=True):
    nc.tensor.matmul(out, lhsT, rhs, start=start, stop=stop)


def split_wide_waits(nc, max_waits=1):
    """walrus in this container supports one sync-wait per instruction;
    move extras onto preceding no-ops on the same engine."""
    for f in nc.m.functions:
        for blk in f.blocks:
            new_insts = []
            for ins in blk.instructions:
                si = ins.sync_info
                if si is not None and si.on_wait and len(si.on_wait) > max_waits:
                    waits = list(si.on_wait)
                    k = 0
                    while len(waits) > max_waits:
                        chunk, waits = waits[:max_waits], waits[max_waits:]
                        nop = mybir.InstNoOp(
                            name=f"{ins.name}-ws{k}", ins=[], outs=[]
                        )
                        nop.engine = ins.engine
                        nop.sync_info = bass_rust.SyncInfo(
                            on_wait=chunk, on_update=[]
                        )
                        new_insts.append(nop)
                        k += 1
                    ins.sync_info = bass_rust.SyncInfo(
                        on_wait=waits, on_update=list(si.on_update or [])
                    )
                new_insts.append(ins)
            blk.instructions = new_insts


G = 32          # front guard columns in the padded volume tile


def _win(pad, tap, q):
    """Window AP into padded volume for depthwise tap, quarter q."""
    kd, r = divmod(tap, 9)
    kh, kw = divmod(r, 3)
    v = pad[:, G : G + PL].rearrange("c (x y z) -> c x y z",
                                     x=PX, y=PX, z=PX)
    return v[:, kd + 4 * q : kd + 4 * q + 4, kh : kh + 16, kw : kw + 16]


def _interior(pad, q):
    v = pad[:, G : G + PL].rearrange("c (x y z) -> c x y z",
                                     x=PX, y=PX, z=PX)
    return v[:, 1 + 4 * q : 5 + 4 * q, 1:17, 1:17]


def build_module():
    nc = bass.Bass("TRN2", target_bir_lowering=False, debug=False)

    vols_d = nc.dram_tensor("vols", [5, C, L], F16, kind="ExternalInput")
    w1t_d = nc.dram_tensor("w1t", [C, NBLK * C], F16, kind="ExternalInput")
    w2t_d = nc.dram_tensor("w2t", [C, NBLK * C], F16, kind="ExternalInput")
    bcol_d = nc.dram_tensor("bcol", [C, NBLK * 3], F32, kind="ExternalInput")
    gcol_d = nc.dram_tensor("gcol", [C, NBLK * 3], F32, kind="ExternalInput")
    bcolb_d = nc.dram_tensor("bcolb", [C, NBLK * 3], F32,
                             kind="ExternalInput")
    dwt_d = nc.dram_tensor("dwt", [C, NBLK * 27], F32, kind="ExternalInput")
    dwdiag_d = nc.dram_tensor(
        "dwdiag", [NBLK, C, NPE * C], F16, kind="ExternalInput"
    )
    identb_d = nc.dram_tensor("identb", [C, C], F16, kind="ExternalInput")
    mask_d = nc.dram_tensor("maskrow", [1, 4 * C], F16, kind="ExternalInput")
    out_d = nc.dram_tensor("out", [C, L], F16, kind="ExternalOutput")

    with tile.TileContext(nc) as tc:
        ctx = contextlib.ExitStack()
        with ctx:
            ctx.enter_context(nc.allow_low_precision(
                reason="fp16 activations; LN stats and matmuls accumulate "
                       "in fp32 PSUM"))
            csts = ctx.enter_context(tc.tile_pool(name="csts", bufs=1))
            volp = ctx.enter_context(tc.tile_pool(name="volp", bufs=2))
            scr = ctx.enter_context(tc.tile_pool(name="scr", bufs=1))
            obfp = ctx.enter_context(tc.tile_pool(name="obfp", bufs=1))
            accp = ctx.enter_context(tc.tile_pool(name="accp", bufs=2))
            padp = ctx.enter_context(tc.tile_pool(name="padp", bufs=1))
            diagp = ctx.enter_context(tc.tile_pool(name="diagp", bufs=1))
            smal = ctx.enter_context(tc.tile_pool(name="smal", bufs=1))
            psum = ctx.enter_context(
                tc.tile_pool(name="psum", bufs=1, space="PSUM")
            )

            # ---- persistent constants ----
            w1t = csts.tile([C, NBLK * C], F16)
            w2t = csts.tile([C, NBLK * C], F16)
            bcol = csts.tile([C, NBLK * 3], F32)
            bcolb = csts.tile([C, NBLK * 3], F32)
            gcol = csts.tile([C, NBLK * 3], F32)
            dwt = csts.tile([C, NBLK * 27], F32)
            identb = csts.tile([C, C], F16)
            oos = csts.tile([C, 1], F16)
            onesr = csts.tile([1, C], F16)
            urow = csts.tile([2, L], F16)    # p0=u, p1=vu (per-token rows)
            maskr = csts.tile([1, 4 * C], F16)
            qT = csts.tile([C, 32 * C], F16)
            lgacc = csts.tile([C, 4 * C], F32)
            av = csts.tile([C, L], F16)
            attnb = csts.tile([C, 4 * C], F16)
            attnT = csts.tile([C, 4 * C], F16)
            ubc0 = csts.tile([C, L], F16)
            ubc1 = csts.tile([C, L], F16)
            vbc0 = csts.tile([C, L], F16)
            vbc1 = csts.tile([C, L], F16)
            ubc = [ubc0, ubc1]
            vbc = [vbc0, vbc1]

            nc.sync.dma_start(out=w1t[:], in_=w1t_d.ap())
            nc.sync.dma_start(out=w2t[:], in_=w2t_d.ap())
            nc.sync.dma_start(out=bcol[:], in_=bcol_d.ap())
            nc.sync.dma_start(out=bcolb[:], in_=bcolb_d.ap())
            nc.sync.dma_start(out=gcol[:], in_=gcol_d.ap())
            nc.sync.dma_start(out=dwt[:], in_=dwt_d.ap())
            nc.sync.dma_start(out=identb[:], in_=identb_d.ap())
            nc.sync.dma_start(out=maskr[:], in_=mask_d.ap())
            nc.vector.memset(oos[:], 1.0 / 128.0)
            nc.vector.memset(onesr[:], 1.0)

            # two persistent zero-padded dw input volumes (ping-pong)
            pads = []
            for i in range(2):
                p = padp.tile([C, PL + 64], F16, tag=f"pad{i}")
                nc.vector.memset(p[:], 0.0)
                pads.append(p)

            def newton_rsqrt(par, y, v, hs):
                """y = 1/sqrt(v), fp32 [C,32] tiles, all on DVE."""
                ta = smal.tile([32, 128], F32, tag=f"nta{par}")
                nc.vector.tensor_scalar(hs[:], v[:], -0.5, None, OP.mult)
                yi = y[:].bitcast(I32)
                nc.vector.tensor_scalar(
                    yi, v[:].bitcast(I32), 1, None, OP.logical_shift_right
                )
                nc.vector.tensor_scalar(yi, yi, -1, None, OP.bitwise_xor)
                nc.vector.tensor_scalar(yi, yi, 0x5F3759E0, None, OP.add)
                for _ in range(2):
                    nc.vector.tensor_mul(ta[:], y[:], y[:])
                    nc.vector.tensor_mul(ta[:], ta[:], hs[:])
                    nc.vector.tensor_scalar(ta[:], ta[:], 1.5, None, OP.add)
                    nc.vector.tensor_mul(y[:], y[:], ta[:])

            def ln_gen(par, x_sb, blki, lnj, dst_of, gelu):
                """LN over channels. x_sb [C,L] fp16, bias already folded
                in. dst_of(q) -> output AP for quarter q."""
                g_ap = gcol[:, blki * 3 + lnj : blki * 3 + lnj + 1]
                be_ap = bcolb[:, blki * 3 + lnj : blki * 3 + lnj + 1]

                stats = smal.tile([32, 256], F32, tag=f"stats{par}")
                for hv in range(2):
                    stq = scr.tile([33, 2 * QL], F32, tag="stq", bufs=2)
                    for qq in range(2):
                        q = 2 * hv + qq
                        qs = slice(q * QL, (q + 1) * QL)
                        sq = scr.tile([C, QL], F16, tag=f"t{par}", bufs=2)
                        nc.vector.tensor_tensor(
                            sq[:], x_sb[:, qs], x_sb[:, qs], OP.mult
                        )
                        st = psum.tile([33, QL], F32, tag=f"s{par}")
                        if KSIM:
                            nc.vector.memset(st[:], 0.0)
                        for h in range(2):
                            hs = slice(q * QL + h * 512,
                                       q * QL + (h + 1) * 512)
                            ho = slice(h * 512, (h + 1) * 512)
                            _mm(nc, st[0:1, ho], oos[:], x_sb[:, hs])
                            _mm(nc, st[32:33, ho], oos[:],
                                sq[:, h * 512 : (h + 1) * 512])
                        qo = slice(qq * QL, (qq + 1) * QL)
                        nc.scalar.copy(stq[:, qo], st[:])
                        yield
                    js = slice(16 * hv, 16 * hv + 16)
                    nc.sync.dma_start(out=stats[js, 0:128], in_=stq[0:1, :])
                    nc.sync.dma_start(out=stats[js, 128:256],
                                      in_=stq[32:33, :])
                yield

                mean = stats[:, 0:128]
                var = smal.tile([32, 128], F32, tag=f"f0{par}")
                hs = smal.tile([32, 128], F32, tag=f"f1{par}")
                y = smal.tile([32, 128], F32, tag=f"f2{par}")
                u = smal.tile([32, 128], F16, tag=f"f3{par}")
                vun = smal.tile([32, 128], F16, tag=f"f4{par}")
                nc.vector.tensor_mul(var[:], mean, mean)
                nc.vector.scalar_tensor_tensor(
                    var[:], stats[:, 128:256], EPS, var[:], OP.add,
                    OP.subtract
                )
                newton_rsqrt(par, y, var, hs)
                nc.vector.tensor_copy(u[:], y[:])
                nc.vector.scalar_tensor_tensor(
                    vun[:], mean, -1.0, y[:], OP.mult, OP.mult
                )
                nc.sync.dma_start(out=urow[0:1, :], in_=u[:])
                nc.sync.dma_start(out=urow[1:2, :], in_=vun[:])
                u_bc, vu_bc = ubc[par], vbc[par]
                for r, bc in ((0, u_bc), (1, vu_bc)):
                    rap = urow[r : r + 1, :].rearrange(
                        "a (b j) -> a b j", b=1
                    ).broadcast_to([1, C, L])
                    nc.sync.dma_start(out=bc[:], in_=rap)
                yield

                for q in range(NQ):
                    qs = slice(q * QL, (q + 1) * QL)
                    pre = scr.tile([C, QL], F16, tag=f"t{par}", bufs=2)
                    nc.vector.tensor_tensor(
                        pre[:], x_sb[:, qs], u_bc[:, qs], OP.mult
                    )
                    nc.vector.tensor_tensor(
                        pre[:], pre[:], vu_bc[:, qs], OP.add
                    )
                    nc.scalar.activation(
                        dst_of(q), pre[:], AF.Gelu if gelu else AF.Identity,
                        bias=be_ap, scale=g_ap
                    )
                    yield

            def pw_gen(par, w_ap, rhs_sb, dst, b_ap):
                for q in range(NQ):
                    xq = psum.tile([C, QL], F32, tag=f"x{par}")
                    for h in range(2):
                        hs = slice(q * QL + h * 512, q * QL + (h + 1) * 512)
                        _mm(nc, xq[:, h * 512 : (h + 1) * 512], w_ap,
                            rhs_sb[:, hs])
                    nc.scalar.activation(
                        dst[:, q * QL : (q + 1) * QL], xq[:], AF.Identity,
                        bias=b_ap, scale=1.0
                    )
                    yield

            def block_gen(blki, vol_idx, dst_of):
                """Full DepthWiseConvBlock as a generator."""
                par = blki % 2
                pad = pads[par]
                vol = volp.tile([C, L], F16, tag="vol")
                nc.sync.dma_start(out=vol[:], in_=vols_d.ap()[vol_idx, :, :])
                diag = diagp.tile([C, NPE * C], F16)
                nc.sync.dma_start(out=diag[:], in_=dwdiag_d.ap()[blki, :, :])
                yield

                w1 = w1t[:, blki * C : (blki + 1) * C]
                w2 = w2t[:, blki * C : (blki + 1) * C]
                b1 = bcol[:, blki * 3 + 0 : blki * 3 + 1]
                b2 = bcol[:, blki * 3 + 1 : blki * 3 + 2]
                b3 = bcol[:, blki * 3 + 2 : blki * 3 + 3]
                x1 = scr.tile([C, L], F16, tag=f"x{par}", bufs=2)
                yield from pw_gen(par, w1, vol, x1, b1)
                yield from ln_gen(par, x1, blki, 0,
                                  lambda q: _interior(pad, q), gelu=True)

                # depthwise: per-quarter PE diag-matmul chain + DVE/Pool
                # fused mul-add chain into a dense accumulator
                x2 = scr.tile([C, L], F16, tag=f"x{par}", bufs=2)
                for q in range(NQ):
                    # 4 padded x-slabs of quarter q (incl. y/z borders)
                    bq = G + (1 + 4 * q) * 324
                    acc = accp.tile([C, 4 * 324], F16, tag=f"ac{par}",
                                    bufs=2)
                    for ti, (t, eng) in enumerate(VEC_TAPS):
                        e = nc.vector if eng == "D" else nc.gpsimd
                        kd, r = divmod(t, 9)
                        kh, kw = divmod(r, 3)
                        dlt = (kd - 1) * 324 + (kh - 1) * 18 + (kw - 1)
                        wcol = dwt[:, blki * 27 + t : blki * 27 + t + 1]
                        srcs = pad[:, bq + dlt : bq + dlt + 4 * 324]
                        if ti == 0:
                            e.tensor_scalar(acc[:], srcs, wcol, None,
                                            OP.mult)
                        else:
                            e.scalar_tensor_tensor(
                                acc[:], srcs, wcol, acc[:], OP.mult, OP.add)
                    accv = acc.rearrange("c (a y z) -> c a y z", a=4, y=18)
                    yield
                    dq = psum.tile([C, QL], F32, tag=f"x{par}")
                    for h in range(2):
                        ho = slice(h * 512, (h + 1) * 512)
                        for ti, t in enumerate(PE_TAPS):
                            w = _win(pad, t, q)
                            wh = w[:, 2 * h : 2 * h + 2, :, :]
                            _mm(nc, dq[:, ho],
                                diag[:, ti * C : (ti + 1) * C], wh,
                                start=(ti == 0), stop=False)
                        _mm(nc, dq[:, ho], identb[:],
                            accv[:, 2 * h : 2 * h + 2, 1:17, 1:17],
                            start=False, stop=True)
                    nc.scalar.activation(
                        x2[:, q * QL : (q + 1) * QL], dq[:], AF.Identity,
                        bias=b2, scale=1.0
                    )
                    yield
                x2g = scr.tile([C, L], F16, tag=f"x{par}", bufs=2)
                yield from ln_gen(par, x2, blki, 1,
                                  lambda q: x2g[:, q * QL : (q + 1) * QL],
                                  gelu=True)

                x3 = scr.tile([C, L], F16, tag=f"x{par}", bufs=2)
                yield from pw_gen(par, w2, x2g, x3, b3)
                yield from ln_gen(par, x3, blki, 2, dst_of, gelu=False)

            def transpose_gen(par, src_bf, dst_ap_of, nj=8):
                """dst_ap_of(j) -> [C, 4, C]-shaped dest AP for l-tiles
                4j..4j+3."""
                for j in range(nj):
                    tp = psum.tile([C, 4 * C], F16, tag=f"x{par}")
                    for t in range(4):
                        li = 4 * j + t
                        nc.tensor.transpose(
                            tp[:, t * C : (t + 1) * C],
                            src_bf[:, li * C : (li + 1) * C], identb[:])
                    nc.scalar.copy(
                        dst_ap_of(j),
                        tp.rearrange("c (a b) -> c a b", a=4))
                    if j % 2 == 1:
                        yield

            # ================= pipelined main program =================
            qTv = qT.rearrange("c (a b) -> c a b", b=C)

            def qblock_gen():
                qbf = obfp.tile([C, L], F16, tag="obf0", bufs=1)
                yield from block_gen(
                    0, 0, lambda q: qbf[:, q * QL : (q + 1) * QL])
                yield from transpose_gen(
                    0, qbf, lambda j: qTv[:, 4 * j : 4 * j + 4, :])

            def kblock_gen(m):
                par = (1 + m) % 2
                kbf = obfp.tile([C, L], F16, tag=f"obf{par}", bufs=1)
                yield from block_gen(
                    1 + m, 1 + m, lambda q: kbf[:, q * QL : (q + 1) * QL])
                lgm = psum.tile([C, C], F32, tag=f"s{par}")
                for ch in range(2):
                    ktmp = scr.tile([C, 16 * C], F16, tag="ktmp", bufs=1)
                    ktv = ktmp.rearrange("c (a b) -> c a b", b=C)
                    yield from transpose_gen(
                        par, kbf[:, ch * 16 * C : (ch + 1) * 16 * C],
                        lambda j: ktv[:, 4 * j : 4 * j + 4, :], nj=4)
                    for i in range(16):
                        gi = 16 * ch + i
                        _mm(nc, lgm[:], qT[:, gi * C : (gi + 1) * C],
                            ktmp[:, i * C : (i + 1) * C],
                            start=(gi == 0), stop=(gi == 31))
                    yield
                nc.vector.tensor_copy(lgacc[:, m * C : (m + 1) * C], lgm[:])
                yield

            def softmax_gen():
                mk = psum.tile([C, 4 * C], F32, tag="s1")
                _mm(nc, mk[:], onesr[0:1, :], maskr[:])
                nc.vector.tensor_scalar_mul(lgacc[:], lgacc[:],
                                            float(L) ** -0.5)
                nc.vector.tensor_add(lgacc[:], lgacc[:], mk[:])
                mx = smal.tile([C, 1], F32, tag="g0")
                nc.vector.tensor_reduce(mx[:], lgacc[:], AX.X, OP.max)
                nc.vector.tensor_scalar_sub(lgacc[:], lgacc[:], mx[:])
                nc.scalar.activation(lgacc[:], lgacc[:], AF.Exp)
                sm = smal.tile([C, 1], F32, tag="g1")
                nc.vector.tensor_reduce(sm[:], lgacc[:], AX.X, OP.add)
                nc.vector.reciprocal(sm[:], sm[:])
                nc.vector.tensor_scalar_mul(attnb[:], lgacc[:], sm[:])
                yield
                tp = psum.tile([C, 4 * C], F16, tag="s1")
                for t in range(4):
                    nc.tensor.transpose(
                        tp[:, t * C : (t + 1) * C],
                        attnb[:, t * C : (t + 1) * C], identb[:])
                nc.vector.tensor_copy(attnT[:], tp[:])
                yield

            def vblock_gen(m):
                par = (1 + m) % 2
                vout = obfp.tile([C, L], F16, tag=f"obf{par}", bufs=1)
                yield from block_gen(
                    5 + m, 1 + m, lambda q: vout[:, q * QL : (q + 1) * QL])
                for q in range(NQ):
                    aq = psum.tile([C, QL], F32, tag=f"x{par}")
                    for h in range(2):
                        hs = slice(q * QL + h * 512, q * QL + (h + 1) * 512)
                        _mm(nc, aq[:, h * 512 : (h + 1) * 512],
                            attnT[:, m * C : (m + 1) * C], vout[:, hs])
                    avq = av[:, q * QL : (q + 1) * QL]
                    if m == 0:
                        nc.vector.tensor_copy(avq, aq[:])
                    else:
                        nc.vector.tensor_add(avq, avq, aq[:])
                    yield

            # block 9 reads av instead of a DRAM volume
            def block_gen9(blki, vol_idx, dst_of):
                par = blki % 2
                pad = pads[par]
                diag = diagp.tile([C, NPE * C], F16)
                nc.sync.dma_start(out=diag[:], in_=dwdiag_d.ap()[blki, :, :])
                yield

                w1 = w1t[:, blki * C : (blki + 1) * C]
                w2 = w2t[:, blki * C : (blki + 1) * C]
                b1 = bcol[:, blki * 3 + 0 : blki * 3 + 1]
                b2 = bcol[:, blki * 3 + 1 : blki * 3 + 2]
                b3 = bcol[:, blki * 3 + 2 : blki * 3 + 3]
                x1 = scr.tile([C, L], F16, tag=f"x{par}", bufs=2)
                yield from pw_gen(par, w1, av, x1, b1)
                yield from ln_gen(par, x1, blki, 0,
                                  lambda q: _interior(pad, q), gelu=True)
                x2 = scr.tile([C, L], F16, tag=f"x{par}", bufs=2)
                for q in range(NQ):
                    # 4 padded x-slabs of quarter q (incl. y/z borders)
                    bq = G + (1 + 4 * q) * 324
                    acc = accp.tile([C, 4 * 324], F16, tag=f"ac{par}",
                                    bufs=2)
                    for ti, (t, eng) in enumerate(VEC_TAPS):
                        e = nc.vector if eng == "D" else nc.gpsimd
                        kd, r = divmod(t, 9)
                        kh, kw = divmod(r, 3)
                        dlt = (kd - 1) * 324 + (kh - 1) * 18 + (kw - 1)
                        wcol = dwt[:, blki * 27 + t : blki * 27 + t + 1]
                        srcs = pad[:, bq + dlt : bq + dlt + 4 * 324]
                        if ti == 0:
                            e.tensor_scalar(acc[:], srcs, wcol, None,
                                            OP.mult)
                        else:
                            e.scalar_tensor_tensor(
                                acc[:], srcs, wcol, acc[:], OP.mult, OP.add)
                    accv = acc.rearrange("c (a y z) -> c a y z", a=4, y=18)
                    yield
                    dq = psum.tile([C, QL], F32, tag=f"x{par}")
                    for h in range(2):
                        ho = slice(h * 512, (h + 1) * 512)
                        for ti, t in enumerate(PE_TAPS):
                            w = _win(pad, t, q)
                            wh = w[:, 2 * h : 2 * h + 2, :, :]
                            _mm(nc, dq[:, ho],
                                diag[:, ti * C : (ti + 1) * C], wh,
                                start=(ti == 0), stop=False)
                        _mm(nc, dq[:, ho], identb[:],
                            accv[:, 2 * h : 2 * h + 2, 1:17, 1:17],
                            start=False, stop=True)
                    nc.scalar.activation(
                        x2[:, q * QL : (q + 1) * QL], dq[:], AF.Identity,
                        bias=b2, scale=1.0
                    )
                    yield
                x2g = scr.tile([C, L], F16, tag=f"x{par}", bufs=2)
                yield from ln_gen(par, x2, blki, 1,
                                  lambda q: x2g[:, q * QL : (q + 1) * QL],
                                  gelu=True)
                x3 = scr.tile([C, L], F16, tag=f"x{par}", bufs=2)
                yield from pw_gen(par, w2, x2g, x3, b3)
                yield from ln_gen(par, x3, blki, 2, dst_of, gelu=False)

            def outblock_gen2():
                fout = obfp.tile([C, L], F16, tag="obf1", bufs=1)
                yield from block_gen9(
                    9, None, lambda q: fout[:, q * QL : (q + 1) * QL])
                nc.sync.dma_start(out=out_d.ap(), in_=fout[:])
                yield

            def chain(*gens):
                for g in gens:
                    yield from g

            kdbg = int(os.environ.get("KDBG", "0"))
            if kdbg == 1:
                def dbg_gen():
                    qbf = obfp.tile([C, L], F16, tag="obf0", bufs=1)
                    yield from block_gen(
                        0, 0, lambda q: qbf[:, q * QL : (q + 1) * QL])
                    nc.sync.dma_start(out=out_d.ap(), in_=qbf[:])
                    yield
                queue = [dbg_gen()]
            elif kdbg == 2:
                def dbg2_gen():
                    yield from softmax_gen()
                    nc.sync.dma_start(out=out_d.ap()[:, 0:4 * C],
                                      in_=attnb[:])
                    yield
                queue = [qblock_gen()]
                for m in range(NMOD - 1):
                    queue.append(kblock_gen(m))
                queue.append(chain(kblock_gen(NMOD - 1), dbg2_gen()))
            elif kdbg == 4:
                def dbg4_gen():
                    vout = obfp.tile([C, L], F16, tag="obf1", bufs=1)
                    yield from block_gen(
                        5, 1, lambda q: vout[:, q * QL : (q + 1) * QL])
                    nc.sync.dma_start(out=out_d.ap(), in_=vout[:])
                    yield
                queue = [qblock_gen()]
                for m in range(NMOD - 1):
                    queue.append(kblock_gen(m))
                queue.append(chain(kblock_gen(NMOD - 1), softmax_gen()))
                queue.append(dbg4_gen())
            elif kdbg == 3:
                def dbg3_gen():
                    nc.sync.dma_start(out=out_d.ap(), in_=av[:])
                    yield
                queue = [qblock_gen()]
                for m in range(NMOD - 1):
                    queue.append(kblock_gen(m))
                queue.append(chain(kblock_gen(NMOD - 1), softmax_gen()))
                for m in range(NMOD - 1):
                    queue.append(vblock_gen(m))
                queue.append(chain(vblock_gen(NMOD - 1), dbg3_gen()))
            else:
                queue = [qblock_gen()]
                for m in range(NMOD - 1):
                    queue.append(kblock_gen(m))
                queue.append(chain(kblock_gen(NMOD - 1), softmax_gen()))
                for m in range(NMOD - 1):
                    queue.append(vblock_gen(m))
                queue.append(chain(vblock_gen(NMOD - 1), outblock_gen2()))

            STAG = 18
            active = []   # [generator, yields_taken]
            while active or queue:
                if queue and (not active or
                              (len(active) < 2 and active[-1][1] >= STAG)):
                    active.append([queue.pop(0), 0])
                for ent in list(active):
                    try:
                        next(ent[0])
                        ent[1] += 1
                    except StopIteration:
                        active.remove(ent)

    split_wide_waits(nc)
    return nc


_CACHED = {}
_RUN_KWARGS = {}
_LAST_RESULT = None


def _build():
    if "nc" not in _CACHED:
        _CACHED["nc"] = build_module()
    return _CACHED["nc"]


def _f16(x):
    return np.asarray(x, dtype=np.float16)


def prepare(**inputs):
    """Build (nc, in_maps) without running. Shared by kernel() and sim.py."""
    query = np.asarray(inputs["query"], np.float32)
    mods = [np.asarray(inputs[k], np.float32)
            for k in ("flair", "t1ce", "t1", "t2")]
    mask = np.asarray(inputs["mask"])
    p = {k: np.asarray(inputs[k], np.float32) for k in
         ("pw1_w", "pw1_b", "ln1_g", "ln1_b", "dw_w", "dw_b",
          "ln2_g", "ln2_b", "pw2_w", "pw2_b", "ln3_g", "ln3_b")}

    w1t = _f16(p["pw1_w"].transpose(2, 0, 1).reshape(C, NBLK * C))
    w2t = _f16(p["pw2_w"].transpose(2, 0, 1).reshape(C, NBLK * C))
    bst = np.stack([p["pw1_b"], p["dw_b"], p["pw2_b"]], axis=1)  # [10,3,C]
    gst = np.stack([p["ln1_g"], p["ln2_g"], p["ln3_g"]], axis=1)
    best = np.stack([p["ln1_b"], p["ln2_b"], p["ln3_b"]], axis=1)
    bcol = np.ascontiguousarray(bst.transpose(2, 0, 1).reshape(C, NBLK * 3))
    gcol = np.ascontiguousarray(gst.transpose(2, 0, 1).reshape(C, NBLK * 3))
    bcolb = np.ascontiguousarray(
        best.transpose(2, 0, 1).reshape(C, NBLK * 3))
    dwt = p["dw_w"].reshape(NBLK, C, 27)
    dwtc = np.ascontiguousarray(
        dwt.transpose(1, 0, 2).reshape(C, NBLK * 27))
    dwdiag = np.zeros((NBLK, C, NPE, C), np.float32)
    ii = np.arange(C)
    for k, t in enumerate(PE_TAPS):
        dwdiag[:, ii, k, ii] = dwt[:, :, t]
    dwdiag = _f16(dwdiag.reshape(NBLK, C, NPE * C))
    identb = _f16(np.eye(C, dtype=np.float32))

    nc = _build()

    shared = dict(w1t=w1t, w2t=w2t, bcol=bcol, bcolb=bcolb, gcol=gcol,
                  dwt=dwtc, dwdiag=dwdiag, identb=identb)
    in_maps = []
    for b in range(B):
        vols = _f16(np.stack(
            [query[b].reshape(C, L)] + [m[b].reshape(C, L) for m in mods]))
        mrow = _f16(np.where(np.repeat(mask[b] > 0, C), 0.0,
                             -60000.0)[None, :])
        in_maps.append(dict(vols=vols, maskrow=mrow, **shared))
    return nc, in_maps


def postprocess_one(out):
    return np.asarray(out, np.float32).reshape(C, S, S, S)


def kernel(**inputs):
    global _LAST_RESULT
    nc, in_maps = prepare(**inputs)
    res = run_bass_kernel_spmd(nc, in_maps, core_ids=list(range(B)),
                               **_RUN_KWARGS)
    _LAST_RESULT = res
    query = np.asarray(inputs["query"], np.float32)
    x = np.stack([postprocess_one(res.results[b]["out"]) for b in range(B)])
    return (query + x).astype(np.float32)


# revision 39
# speedup vs baseline: 2.4549x; 2.4549x over previous
"""Trainium2 Bass kernel for nn_Decoder_fusion (sparse_attention).

Data-parallel over batch B=8 across 8 NeuronCores (one sample per core).
Per-core layout: channel-major [C=128 partitions, L=4096 tokens], fp16
activations (fp32 PSUM accumulation everywhere).

Two dwblocks run software-pipelined: each block is emitted by a
generator that yields between micro-phases, and a round-robin driver
interleaves two blocks' instruction streams so the in-order engines
overlap block n's vector/DMA phases with block n+1's PE phases.
All transient buffers are parity-tagged (blki % 2).

Per dwblock:
  pw conv   -> PE matmuls; PSUM->SBUF copy on ACT folds the conv bias
  LayerNorm -> per-token stats via PE ones-reduction matmuls (Square on
               DVE); stats rows staged by ACT, reshaped by DMA; rstd via
               DVE Newton rsqrt (no ACT table swaps); u/-mu*u rows
               broadcast to all partitions by DMA; apply is two fp16 2x
               DVE tensor_tensor passes; gamma/beta ride the ACT
               Gelu/Identity pass as per-partition scale/bias
  depthwise -> 27 taps: PE diag-matmuls into PSUM plus a per-quarter
               DVE/Pool fused mul-add chain into a dense accumulator
               merged through the PE
Attention: q/K transposed tile-wise on PE; per-K-block logits matmuls
accumulate into an SBUF fp32 tile; masked softmax on a [128,512] tile;
attn@V per modality accumulated into SBUF by DVE.
The fp32 residual (query + x) is added on the host.
"""

import os
import sys

sys.path.insert(0, "/opt/trn_rl_repo")

import contextlib

import numpy as np

import bass_rust
import concourse.bass as bass
import concourse.mybir as mybir
import concourse.tile as tile
from concourse.bass_utils import run_bass_kernel_spmd

# Old walrus encodes EVENT_SEMAPHORE_RANGE_CLEAR / drain-reset ranges of at
# most 9 semaphores; cap the ranges bass emits at tile-context exit.
_orig_ctr = bass.compact_to_ranges


def _capped_ctr(vals):
    out = []
    for r in _orig_ctr(vals):
        vs = list(r)
        for i in range(0, len(vs), 9):
            chunk = vs[i : i + 9]
            out.append(range(chunk[0], chunk[-1] + 1))
    return out


bass.compact_to_ranges = _capped_ctr

F32 = mybir.dt.float32
F16 = mybir.dt.float16
I32 = mybir.dt.int32
AF = mybir.ActivationFunctionType
OP = mybir.AluOpType
AX = mybir.AxisListType

KSIM = bool(int(os.environ.get("KSIM", "0")))
B, C, S = 8, 128, 16
L = S * S * S            # 4096
PX = S + 2               # 18
PL = PX * PX * PX        # 5832
NBLK = 10
NMOD = 4
NQ = 4                   # quarters per volume
QL = L // NQ             # 1024 tokens per quarter
EPS = 1e-6

# Depthwise tap split: PE diag-matmuls vs DVE/Pool fused mul-add chain.
# VEC_TAPS entries are (tap, engine) with engine "D" (DVE) or "P" (Pool).
VEC_TAPS = ((4, "D"), (13, "D"), (22, "D"), (10, "D"), (16, "D"))
PE_TAPS = tuple(t for t in range(27)
                if t not in tuple(v[0] for v in VEC_TAPS))
NPE = len(PE_TAPS)


def _mm(nc, out, lhsT, rhs, start=True, stop=True):
    nc.tensor.matmul(out, lhsT, rhs, start=start, stop=stop)


def split_wide_waits(nc, max_waits=1):
    """walrus in this container supports one sync-wait per instruction;
    move extras onto preceding no-ops on the same engine."""
    for f in nc.m.functions:
        for blk in f.blocks:
            new_insts = []
            for ins in blk.instructions:
                si = ins.sync_info
                if si is not None and si.on_wait and len(si.on_wait) > max_waits:
                    waits = list(si.on_wait)
                    k = 0
                    while len(waits) > max_waits:
                        chunk, waits = waits[:max_waits], waits[max_waits:]
                        nop = mybir.InstNoOp(
                            name=f"{ins.name}-ws{k}", ins=[], outs=[]
                        )
                        nop.engine = ins.engine
                        nop.sync_info = bass_rust.SyncInfo(
                            on_wait=chunk, on_update=[]
                        )
                        new_insts.append(nop)
                        k += 1
                    ins.sync_info = bass_rust.SyncInfo(
                        on_wait=waits, on_update=list(si.on_update or [])
                    )
                new_insts.append(ins)
            blk.instructions = new_insts


G = 32          # front guard columns in the padded volume tile


def _win(pad, tap, q):
    """Window AP into padded volume for depthwise tap, quarter q."""
    kd, r = divmod(tap, 9)
    kh, kw = divmod(r, 3)
    v = pad[:, G : G + PL].rearrange("c (x y z) -> c x y z",
                                     x=PX, y=PX, z=PX)
    return v[:, kd + 4 * q : kd + 4 * q + 4, kh : kh + 16, kw : kw + 16]


def _interior(pad, q):
    v = pad[:, G : G + PL].rearrange("c (x y z) -> c x y z",
                                     x=PX, y=PX, z=PX)
    return v[:, 1 + 4 * q : 5 + 4 * q, 1:17, 1:17]


def build_module():
    nc = bass.Bass("TRN2", target_bir_lowering=False, debug=False)

    vols_d = nc.dram_tensor("vols", [5, C, L], F16, kind="ExternalInput")
    w1t_d = nc.dram_tensor("w1t", [C, NBLK * C], F16, kind="ExternalInput")
    w2t_d = nc.dram_tensor("w2t", [C, NBLK * C], F16, kind="ExternalInput")
    bcol_d = nc.dram_tensor("bcol", [C, NBLK * 3], F32, kind="ExternalInput")
    gcol_d = nc.dram_tensor("gcol", [C, NBLK * 3], F32, kind="ExternalInput")
    bcolb_d = nc.dram_tensor("bcolb", [C, NBLK * 3], F32,
                             kind="ExternalInput")
    dwt_d = nc.dram_tensor("dwt", [C, NBLK * 27], F32, kind="ExternalInput")
    dwdiag_d = nc.dram_tensor(
        "dwdiag", [NBLK, C, NPE * C], F16, kind="ExternalInput"
    )
    identb_d = nc.dram_tensor("identb", [C, C], F16, kind="ExternalInput")
    mask_d = nc.dram_tensor("maskrow", [1, 4 * C], F16, kind="ExternalInput")
    out_d = nc.dram_tensor("out", [C, L], F16, kind="ExternalOutput")

    with tile.TileContext(nc) as tc:
        ctx = contextlib.ExitStack()
        with ctx:
            ctx.enter_context(nc.allow_low_precision(
                reason="fp16 activations; LN stats and matmuls accumulate "
                       "in fp32 PSUM"))
            csts = ctx.enter_context(tc.tile_pool(name="csts", bufs=1))
            volp = ctx.enter_context(tc.tile_pool(name="volp", bufs=2))
            scr = ctx.enter_context(tc.tile_pool(name="scr", bufs=1))
            obfp = ctx.enter_context(tc.tile_pool(name="obfp", bufs=1))
            accp = ctx.enter_context(tc.tile_pool(name="accp", bufs=2))
            padp = ctx.enter_context(tc.tile_pool(name="padp", bufs=1))
            diagp = ctx.enter_context(tc.tile_pool(name="diagp", bufs=2))
            smal = ctx.enter_context(tc.tile_pool(name="smal", bufs=1))
            psum = ctx.enter_context(
                tc.tile_pool(name="psum", bufs=1, space="PSUM")
            )

            # ---- persistent constants ----
            w1t = csts.tile([C, NBLK * C], F16)
            w2t = csts.tile([C, NBLK * C], F16)
            bcol = csts.tile([C, NBLK * 3], F32)
            bcolb = csts.tile([C, NBLK * 3], F32)
            gcol = csts.tile([C, NBLK * 3], F32)
            dwt = csts.tile([C, NBLK * 27], F32)
            identb = csts.tile([C, C], F16)
            oos = csts.tile([C, 1], F16)
            onesr = csts.tile([33, C], F16)
            urow0 = csts.tile([33, L], F16)  # p0=u, p32=vu (per-token rows)
            urow1 = csts.tile([33, L], F16)
            urows = [urow0, urow1]
            maskr = csts.tile([1, 4 * C], F16)
            qT = csts.tile([C, 32 * C], F16)
            lgacc = csts.tile([C, 4 * C], F32)
            av = csts.tile([C, L], F16)
            attnb = csts.tile([C, 4 * C], F16)
            attnT = csts.tile([C, 4 * C], F16)

            nc.sync.dma_start(out=w1t[:], in_=w1t_d.ap())
            nc.sync.dma_start(out=w2t[:], in_=w2t_d.ap())
            nc.sync.dma_start(out=bcol[:], in_=bcol_d.ap())
            nc.sync.dma_start(out=bcolb[:], in_=bcolb_d.ap())
            nc.sync.dma_start(out=gcol[:], in_=gcol_d.ap())
            nc.sync.dma_start(out=dwt[:], in_=dwt_d.ap())
            nc.sync.dma_start(out=identb[:], in_=identb_d.ap())
            nc.sync.dma_start(out=maskr[:], in_=mask_d.ap())
            nc.vector.memset(oos[:], 1.0 / 128.0)
            nc.vector.memset(onesr[:], 1.0)

            # two persistent zero-padded dw input volumes (ping-pong)
            pads = []
            for i in range(2):
                p = padp.tile([C, PL + 64], F16, tag=f"pad{i}")
                nc.vector.memset(p[:], 0.0)
                pads.append(p)

            def newton_rsqrt(par, y, v, hs):
                """y = 1/sqrt(v), fp32 [C,32] tiles, all on DVE."""
                ta = smal.tile([32, 128], F32, tag=f"nta{par}")
                nc.vector.tensor_scalar(hs[:], v[:], -0.5, None, OP.mult)
                yi = y[:].bitcast(I32)
                nc.vector.tensor_scalar(
                    yi, v[:].bitcast(I32), 1, None, OP.logical_shift_right
                )
                nc.vector.tensor_scalar(yi, yi, -1, None, OP.bitwise_xor)
                nc.vector.tensor_scalar(yi, yi, 0x5F3759E0, None, OP.add)
                for _ in range(2):
                    nc.vector.tensor_mul(ta[:], y[:], y[:])
                    nc.vector.tensor_mul(ta[:], ta[:], hs[:])
                    nc.vector.tensor_scalar(ta[:], ta[:], 1.5, None, OP.add)
                    nc.vector.tensor_mul(y[:], y[:], ta[:])

            def ln_gen(par, x_sb, blki, lnj, dst_of, gelu):
                """LN over channels. x_sb [C,L] fp16, bias already folded
                in. dst_of(q) -> output AP for quarter q."""
                g_ap = gcol[:, blki * 3 + lnj : blki * 3 + lnj + 1]
                be_ap = bcolb[:, blki * 3 + lnj : blki * 3 + lnj + 1]

                stats = smal.tile([32, 256], F32, tag=f"stats{par}")
                for hv in range(2):
                    stq = scr.tile([33, 2 * QL], F32, tag="stq", bufs=2)
                    for qq in range(2):
                        q = 2 * hv + qq
                        qs = slice(q * QL, (q + 1) * QL)
                        sq = scr.tile([C, QL], F16, tag=f"t{par}", bufs=2)
                        nc.vector.tensor_tensor(
                            sq[:], x_sb[:, qs], x_sb[:, qs], OP.mult
                        )
                        st = psum.tile([33, QL], F32, tag=f"s{par}")
                        if KSIM:
                            nc.vector.memset(st[:], 0.0)
                        for h in range(2):
                            hs = slice(q * QL + h * 512,
                                       q * QL + (h + 1) * 512)
                            ho = slice(h * 512, (h + 1) * 512)
                            _mm(nc, st[0:1, ho], oos[:], x_sb[:, hs])
                            _mm(nc, st[32:33, ho], oos[:],
                                sq[:, h * 512 : (h + 1) * 512])
                        qo = slice(qq * QL, (qq + 1) * QL)
                        nc.scalar.copy(stq[:, qo], st[:])
                        yield
                    js = slice(16 * hv, 16 * hv + 16)
                    nc.sync.dma_start(out=stats[js, 0:128], in_=stq[0:1, :])
                    nc.sync.dma_start(out=stats[js, 128:256],
                                      in_=stq[32:33, :])
                yield

                mean = stats[:, 0:128]
                var = smal.tile([32, 128], F32, tag=f"f0{par}")
                hs = smal.tile([32, 128], F32, tag=f"f1{par}")
                y = smal.tile([32, 128], F32, tag=f"f2{par}")
                u = smal.tile([32, 128], F16, tag=f"f3{par}")
                vun = smal.tile([32, 128], F16, tag=f"f4{par}")
                nc.vector.tensor_mul(var[:], mean, mean)
                nc.vector.scalar_tensor_tensor(
                    var[:], stats[:, 128:256], EPS, var[:], OP.add,
                    OP.subtract
                )
                newton_rsqrt(par, y, var, hs)
                nc.vector.tensor_copy(u[:], y[:])
                nc.vector.scalar_tensor_tensor(
                    vun[:], mean, -1.0, y[:], OP.mult, OP.mult
                )
                urow = urows[par]
                nc.sync.dma_start(out=urow[0:1, :], in_=u[:])
                nc.sync.dma_start(out=urow[32:33, :], in_=vun[:])
                yield

                for q in range(NQ):
                    qs = slice(q * QL, (q + 1) * QL)
                    ug = psum.tile([C, QL], F32, tag=f"x{par}")
                    vg = psum.tile([C, QL], F32, tag=f"s{par}")
                    for h in range(2):
                        hs2 = slice(q * QL + h * 512,
                                    q * QL + (h + 1) * 512)
                        ho = slice(h * 512, (h + 1) * 512)
                        _mm(nc, ug[:, ho], onesr[0:1, :], urow[0:1, hs2])
                        _mm(nc, vg[:, ho], onesr[32:33, :],
                            urow[32:33, hs2])
                    pre = scr.tile([C, QL], F16, tag=f"t{par}", bufs=2)
                    nc.vector.tensor_tensor(
                        pre[:], x_sb[:, qs], ug[:], OP.mult
                    )
                    nc.vector.tensor_tensor(
                        pre[:], pre[:], vg[:], OP.add
                    )
                    nc.scalar.activation(
                        dst_of(q), pre[:], AF.Gelu if gelu else AF.Identity,
                        bias=be_ap, scale=g_ap
                    )
                    yield

            def pw_gen(par, w_ap, rhs_sb, dst, b_ap):
                for q in range(NQ):
                    xq = psum.tile([C, QL], F32, tag=f"x{par}")
                    for h in range(2):
                        hs = slice(q * QL + h * 512, q * QL + (h + 1) * 512)
                        _mm(nc, xq[:, h * 512 : (h + 1) * 512], w_ap,
                            rhs_sb[:, hs])
                    nc.scalar.activation(
                        dst[:, q * QL : (q + 1) * QL], xq[:], AF.Identity,
                        bias=b_ap, scale=1.0
                    )
                    yield

            def block_gen(blki, vol_idx, dst_of):
                """Full DepthWiseConvBlock as a generator."""
                par = blki % 2
                pad = pads[par]
                vol = volp.tile([C, L], F16, tag="vol")
                nc.sync.dma_start(out=vol[:], in_=vols_d.ap()[vol_idx, :, :])
                diag = diagp.tile([C, NPE * C], F16)
                nc.sync.dma_start(out=diag[:], in_=dwdiag_d.ap()[blki, :, :])
                yield

                w1 = w1t[:, blki * C : (blki + 1) * C]
                w2 = w2t[:, blki * C : (blki + 1) * C]
                b1 = bcol[:, blki * 3 + 0 : blki * 3 + 1]
                b2 = bcol[:, blki * 3 + 1 : blki * 3 + 2]
                b3 = bcol[:, blki * 3 + 2 : blki * 3 + 3]
                x1 = scr.tile([C, L], F16, tag=f"x{par}", bufs=2)
                yield from pw_gen(par, w1, vol, x1, b1)
                yield from ln_gen(par, x1, blki, 0,
                                  lambda q: _interior(pad, q), gelu=True)

                # depthwise: per-quarter PE diag-matmul chain + DVE/Pool
                # fused mul-add chain into a dense accumulator
                x2 = scr.tile([C, L], F16, tag=f"x{par}", bufs=2)
                for q in range(NQ):
                    # 4 padded x-slabs of quarter q (incl. y/z borders)
                    bq = G + (1 + 4 * q) * 324
                    acc = accp.tile([C, 4 * 324], F16, tag=f"ac{par}",
                                    bufs=2)
                    for ti, (t, eng) in enumerate(VEC_TAPS):
                        e = nc.vector if eng == "D" else nc.gpsimd
                        kd, r = divmod(t, 9)
                        kh, kw = divmod(r, 3)
                        dlt = (kd - 1) * 324 + (kh - 1) * 18 + (kw - 1)
                        wcol = dwt[:, blki * 27 + t : blki * 27 + t + 1]
                        srcs = pad[:, bq + dlt : bq + dlt + 4 * 324]
                        if ti == 0:
                            e.tensor_scalar(acc[:], srcs, wcol, None,
                                            OP.mult)
                        else:
                            e.scalar_tensor_tensor(
                                acc[:], srcs, wcol, acc[:], OP.mult, OP.add)
                    accv = acc.rearrange("c (a y z) -> c a y z", a=4, y=18)
                    yield
                    dq = psum.tile([C, QL], F32, tag=f"x{par}")
                    for h in range(2):
                        ho = slice(h * 512, (h + 1) * 512)
                        for ti, t in enumerate(PE_TAPS):
                            w = _win(pad, t, q)
                            wh = w[:, 2 * h : 2 * h + 2, :, :]
                            _mm(nc, dq[:, ho],
                                diag[:, ti * C : (ti + 1) * C], wh,
                                start=(ti == 0), stop=False)
                        _mm(nc, dq[:, ho], identb[:],
                            accv[:, 2 * h : 2 * h + 2, 1:17, 1:17],
                            start=False, stop=True)
                    nc.scalar.activation(
                        x2[:, q * QL : (q + 1) * QL], dq[:], AF.Identity,
                        bias=b2, scale=1.0
                    )
                    yield
                x2g = scr.tile([C, L], F16, tag=f"x{par}", bufs=2)
                yield from ln_gen(par, x2, blki, 1,
                                  lambda q: x2g[:, q * QL : (q + 1) * QL],
                                  gelu=True)

                x3 = scr.tile([C, L], F16, tag=f"x{par}", bufs=2)
                yield from pw_gen(par, w2, x2g, x3, b3)
                yield from ln_gen(par, x3, blki, 2, dst_of, gelu=False)

            def transpose_gen(par, src_bf, dst_ap_of, nj=8):
                """dst_ap_of(j) -> [C, 4, C]-shaped dest AP for l-tiles
                4j..4j+3."""
                for j in range(nj):
                    tp = psum.tile([C, 4 * C], F16, tag=f"x{par}")
                    for t in range(4):
                        li = 4 * j + t
                        nc.tensor.transpose(
                            tp[:, t * C : (t + 1) * C],
                            src_bf[:, li * C : (li + 1) * C], identb[:])
                    nc.scalar.copy(
                        dst_ap_of(j),
                        tp.rearrange("c (a b) -> c a b", a=4))
                    if j % 2 == 1:
                        yield

            # ================= pipelined main program =================
            qTv = qT.rearrange("c (a b) -> c a b", b=C)

            def qblock_gen():
                qbf = obfp.tile([C, L], F16, tag="obf0", bufs=1)
                yield from block_gen(
                    0, 0, lambda q: qbf[:, q * QL : (q + 1) * QL])
                yield from transpose_gen(
                    0, qbf, lambda j: qTv[:, 4 * j : 4 * j + 4, :])

            def kblock_gen(m):
                par = (1 + m) % 2
                kbf = obfp.tile([C, L], F16, tag=f"obf{par}", bufs=1)
                yield from block_gen(
                    1 + m, 1 + m, lambda q: kbf[:, q * QL : (q + 1) * QL])
                lgm = psum.tile([C, C], F32, tag=f"s{par}")
                for ch in range(2):
                    ktmp = scr.tile([C, 16 * C], F16, tag="ktmp", bufs=1)
                    ktv = ktmp.rearrange("c (a b) -> c a b", b=C)
                    yield from transpose_gen(
                        par, kbf[:, ch * 16 * C : (ch + 1) * 16 * C],
                        lambda j: ktv[:, 4 * j : 4 * j + 4, :], nj=4)
                    for i in range(16):
                        gi = 16 * ch + i
                        _mm(nc, lgm[:], qT[:, gi * C : (gi + 1) * C],
                            ktmp[:, i * C : (i + 1) * C],
                            start=(gi == 0), stop=(gi == 31))
                    yield
                nc.vector.tensor_copy(lgacc[:, m * C : (m + 1) * C], lgm[:])
                yield

            def softmax_gen():
                mk = psum.tile([C, 4 * C], F32, tag="s1")
                _mm(nc, mk[:], onesr[0:1, :], maskr[:])
                nc.vector.tensor_scalar_mul(lgacc[:], lgacc[:],
                                            float(L) ** -0.5)
                nc.vector.tensor_add(lgacc[:], lgacc[:], mk[:])
                mx = smal.tile([C, 1], F32, tag="g0")
                nc.vector.tensor_reduce(mx[:], lgacc[:], AX.X, OP.max)
                nc.vector.tensor_scalar_sub(lgacc[:], lgacc[:], mx[:])
                nc.scalar.activation(lgacc[:], lgacc[:], AF.Exp)
                sm = smal.tile([C, 1], F32, tag="g1")
                nc.vector.tensor_reduce(sm[:], lgacc[:], AX.X, OP.add)
                nc.vector.reciprocal(sm[:], sm[:])
                nc.vector.tensor_scalar_mul(attnb[:], lgacc[:], sm[:])
                yield
                tp = psum.tile([C, 4 * C], F16, tag="s1")
                for t in range(4):
                    nc.tensor.transpose(
                        tp[:, t * C : (t + 1) * C],
                        attnb[:, t * C : (t + 1) * C], identb[:])
                nc.vector.tensor_copy(attnT[:], tp[:])
                yield

            def vblock_gen(m):
                par = (1 + m) % 2
                vout = obfp.tile([C, L], F16, tag=f"obf{par}", bufs=1)
                yield from block_gen(
                    5 + m, 1 + m, lambda q: vout[:, q * QL : (q + 1) * QL])
                for q in range(NQ):
                    aq = psum.tile([C, QL], F32, tag=f"x{par}")
                    for h in range(2):
                        hs = slice(q * QL + h * 512, q * QL + (h + 1) * 512)
                        _mm(nc, aq[:, h * 512 : (h + 1) * 512],
                            attnT[:, m * C : (m + 1) * C], vout[:, hs])
                    avq = av[:, q * QL : (q + 1) * QL]
                    if m == 0:
                        nc.vector.tensor_copy(avq, aq[:])
                    else:
                        nc.vector.tensor_add(avq, avq, aq[:])
                    yield

            # block 9 reads av instead of a DRAM volume
            def block_gen9(blki, vol_idx, dst_of):
                par = blki % 2
                pad = pads[par]
                diag = diagp.tile([C, NPE * C], F16)
                nc.sync.dma_start(out=diag[:], in_=dwdiag_d.ap()[blki, :, :])
                yield

                w1 = w1t[:, blki * C : (blki + 1) * C]
                w2 = w2t[:, blki * C : (blki + 1) * C]
                b1 = bcol[:, blki * 3 + 0 : blki * 3 + 1]
                b2 = bcol[:, blki * 3 + 1 : blki * 3 + 2]
                b3 = bcol[:, blki * 3 + 2 : blki * 3 + 3]
                x1 = scr.tile([C, L], F16, tag=f"x{par}", bufs=2)
                yield from pw_gen(par, w1, av, x1, b1)
                yield from ln_gen(par, x1, blki, 0,
                                  lambda q: _interior(pad, q), gelu=True)
                x2 = scr.tile([C, L], F16, tag=f"x{par}", bufs=2)
                for q in range(NQ):
                    # 4 padded x-slabs of quarter q (incl. y/z borders)
                    bq = G + (1 + 4 * q) * 324
                    acc = accp.tile([C, 4 * 324], F16, tag=f"ac{par}",
                                    bufs=2)
                    for ti, (t, eng) in enumerate(VEC_TAPS):
                        e = nc.vector if eng == "D" else nc.gpsimd
                        kd, r = divmod(t, 9)
                        kh, kw = divmod(r, 3)
                        dlt = (kd - 1) * 324 + (kh - 1) * 18 + (kw - 1)
                        wcol = dwt[:, blki * 27 + t : blki * 27 + t + 1]
                        srcs = pad[:, bq + dlt : bq + dlt + 4 * 324]
                        if ti == 0:
                            e.tensor_scalar(acc[:], srcs, wcol, None,
                                            OP.mult)
                        else:
                            e.scalar_tensor_tensor(
                                acc[:], srcs, wcol, acc[:], OP.mult, OP.add)
                    accv = acc.rearrange("c (a y z) -> c a y z", a=4, y=18)
                    yield
                    dq = psum.tile([C, QL], F32, tag=f"x{par}")
                    for h in range(2):
                        ho = slice(h * 512, (h + 1) * 512)
                        for ti, t in enumerate(PE_TAPS):
                            w = _win(pad, t, q)
                            wh = w[:, 2 * h : 2 * h + 2, :, :]
                            _mm(nc, dq[:, ho],
                                diag[:, ti * C : (ti + 1) * C], wh,
                                start=(ti == 0), stop=False)
                        _mm(nc, dq[:, ho], identb[:],
                            accv[:, 2 * h : 2 * h + 2, 1:17, 1:17],
                            start=False, stop=True)
                    nc.scalar.activation(
                        x2[:, q * QL : (q + 1) * QL], dq[:], AF.Identity,
                        bias=b2, scale=1.0
                    )
                    yield
                x2g = scr.tile([C, L], F16, tag=f"x{par}", bufs=2)
                yield from ln_gen(par, x2, blki, 1,
                                  lambda q: x2g[:, q * QL : (q + 1) * QL],
                                  gelu=True)
                x3 = scr.tile([C, L], F16, tag=f"x{par}", bufs=2)
                yield from pw_gen(par, w2, x2g, x3, b3)
                yield from ln_gen(par, x3, blki, 2, dst_of, gelu=False)

            def outblock_gen2():
                fout = obfp.tile([C, L], F16, tag="obf1", bufs=1)
                yield from block_gen9(
                    9, None, lambda q: fout[:, q * QL : (q + 1) * QL])
                nc.sync.dma_start(out=out_d.ap(), in_=fout[:])
                yield

            def chain(*gens):
                for g in gens:
                    yield from g

            kdbg = int(os.environ.get("KDBG", "0"))
            if kdbg == 1:
                def dbg_gen():
                    qbf = obfp.tile([C, L], F16, tag="obf0", bufs=1)
                    yield from block_gen(
                        0, 0, lambda q: qbf[:, q * QL : (q + 1) * QL])
                    nc.sync.dma_start(out=out_d.ap(), in_=qbf[:])
                    yield
                queue = [dbg_gen()]
            elif kdbg == 2:
                def dbg2_gen():
                    yield from softmax_gen()
                    nc.sync.dma_start(out=out_d.ap()[:, 0:4 * C],
                                      in_=attnb[:])
                    yield
                queue = [qblock_gen()]
                for m in range(NMOD - 1):
                    queue.append(kblock_gen(m))
                queue.append(chain(kblock_gen(NMOD - 1), dbg2_gen()))
            elif kdbg == 4:
                def dbg4_gen():
                    vout = obfp.tile([C, L], F16, tag="obf1", bufs=1)
                    yield from block_gen(
                        5, 1, lambda q: vout[:, q * QL : (q + 1) * QL])
                    nc.sync.dma_start(out=out_d.ap(), in_=vout[:])
                    yield
                queue = [qblock_gen()]
                for m in range(NMOD - 1):
                    queue.append(kblock_gen(m))
                queue.append(chain(kblock_gen(NMOD - 1), softmax_gen()))
                queue.append(dbg4_gen())
            elif kdbg == 3:
                def dbg3_gen():
                    nc.sync.dma_start(out=out_d.ap(), in_=av[:])
                    yield
                queue = [qblock_gen()]
                for m in range(NMOD - 1):
                    queue.append(kblock_gen(m))
                queue.append(chain(kblock_gen(NMOD - 1), softmax_gen()))
                for m in range(NMOD - 1):
                    queue.append(vblock_gen(m))
                queue.append(chain(vblock_gen(NMOD - 1), dbg3_gen()))
            else:
                queue = [qblock_gen()]
                for m in range(NMOD - 1):
                    queue.append(kblock_gen(m))
                queue.append(chain(kblock_gen(NMOD - 1), softmax_gen()))
                for m in range(NMOD - 1):
                    queue.append(vblock_gen(m))
                queue.append(chain(vblock_gen(NMOD - 1), outblock_gen2()))

            STAG = 18
            active = []   # [generator, yields_taken]
            while active or queue:
                if queue and (not active or
                              (len(active) < 2 and active[-1][1] >= STAG)):
                    active.append([queue.pop(0), 0])
                for ent in list(active):
                    try:
                        next(ent[0])
                        ent[1] += 1
                    except StopIteration:
                        active.remove(ent)

    split_wide_waits(nc)
    return nc


_CACHED = {}
_RUN_KWARGS = {}
_LAST_RESULT = None


def _build():
    if "nc" not in _CACHED:
        _CACHED["nc"] = build_module()
    return _CACHED["nc"]


def _f16(x):
    return np.asarray(x, dtype=np.float16)


def prepare(**inputs):
    """Build (nc, in_maps) without running. Shared by kernel() and sim.py."""
    query = np.asarray(inputs["query"], np.float32)
    mods = [np.asarray(inputs[k], np.float32)
            for k in ("flair", "t1ce", "t1", "t2")]
    mask = np.asarray(inputs["mask"])
    p = {k: np.asarray(inputs[k], np.float32) for k in
         ("pw1_w", "pw1_b", "ln1_g", "ln1_b", "dw_w", "dw_b",
          "ln2_g", "ln2_b", "pw2_w", "pw2_b", "ln3_g", "ln3_b")}

    w1t = _f16(p["pw1_w"].transpose(2, 0, 1).reshape(C, NBLK * C))
    w2t = _f16(p["pw2_w"].transpose(2, 0, 1).reshape(C, NBLK * C))
    bst = np.stack([p["pw1_b"], p["dw_b"], p["pw2_b"]], axis=1)  # [10,3,C]
    gst = np.stack([p["ln1_g"], p["ln2_g"], p["ln3_g"]], axis=1)
    best = np.stack([p["ln1_b"], p["ln2_b"], p["ln3_b"]], axis=1)
    bcol = np.ascontiguousarray(bst.transpose(2, 0, 1).reshape(C, NBLK * 3))
    gcol = np.ascontiguousarray(gst.transpose(2, 0, 1).reshape(C, NBLK * 3))
    bcolb = np.ascontiguousarray(
        best.transpose(2, 0, 1).reshape(C, NBLK * 3))
    dwt = p["dw_w"].reshape(NBLK, C, 27)
    dwtc = np.ascontiguousarray(
        dwt.transpose(1, 0, 2).reshape(C, NBLK * 27))
    dwdiag = np.zeros((NBLK, C, NPE, C), np.float32)
    ii = np.arange(C)
    for k, t in enumerate(PE_TAPS):
        dwdiag[:, ii, k, ii] = dwt[:, :, t]
    dwdiag = _f16(dwdiag.reshape(NBLK, C, NPE * C))
    identb = _f16(np.eye(C, dtype=np.float32))

    nc = _build()

    shared = dict(w1t=w1t, w2t=w2t, bcol=bcol, bcolb=bcolb, gcol=gcol,
                  dwt=dwtc, dwdiag=dwdiag, identb=identb)
    in_maps = []
    for b in range(B):
        vols = _f16(np.stack(
            [query[b].reshape(C, L)] + [m[b].reshape(C, L) for m in mods]))
        mrow = _f16(np.where(np.repeat(mask[b] > 0, C), 0.0,
                             -60000.0)[None, :])
        in_maps.append(dict(vols=vols, maskrow=mrow, **shared))
    return nc, in_maps


def postprocess_one(out):
    return np.asarray(out, np.float32).reshape(C, S, S, S)


def kernel(**inputs):
    global _LAST_RESULT
    nc, in_maps = prepare(**inputs)
    res = run_bass_kernel_spmd(nc, in_maps, core_ids=list(range(B)),
                               **_RUN_KWARGS)
    _LAST_RESULT = res
    query = np.asarray(inputs["query"], np.float32)
    x = np.stack([postprocess_one(res.results[b]["out"]) for b in range(B)])
    return (query + x).astype(np.float32)


# revision 40
# speedup vs baseline: 2.5558x; 1.0411x over previous
"""Trainium2 Bass kernel for nn_Decoder_fusion (sparse_attention).

Data-parallel over batch B=8 across 8 NeuronCores (one sample per core).
Per-core layout: channel-major [C=128 partitions, L=4096 tokens], fp16
activations (fp32 PSUM accumulation everywhere).

Two dwblocks run software-pipelined: each block is emitted by a
generator that yields between micro-phases, and a round-robin driver
interleaves two blocks' instruction streams so the in-order engines
overlap block n's vector/DMA phases with block n+1's PE phases.
All transient buffers are parity-tagged (blki % 2).

Per dwblock:
  pw conv   -> PE matmuls; PSUM->SBUF copy on ACT folds the conv bias
  LayerNorm -> per-token stats via PE ones-reduction matmuls (Square on
               DVE); stats rows staged by ACT, reshaped by DMA; rstd via
               DVE Newton rsqrt (no ACT table swaps); u/-mu*u rows
               broadcast to all partitions by DMA; apply is two fp16 2x
               DVE tensor_tensor passes; gamma/beta ride the ACT
               Gelu/Identity pass as per-partition scale/bias
  depthwise -> 27 taps: PE diag-matmuls into PSUM plus a per-quarter
               DVE/Pool fused mul-add chain into a dense accumulator
               merged through the PE
Attention: q/K transposed tile-wise on PE; per-K-block logits matmuls
accumulate into an SBUF fp32 tile; masked softmax on a [128,512] tile;
attn@V per modality accumulated into SBUF by DVE.
The fp32 residual (query + x) is added on the host.
"""

import os
import sys

sys.path.insert(0, "/opt/trn_rl_repo")

import contextlib

import numpy as np

import bass_rust
import concourse.bass as bass
import concourse.mybir as mybir
import concourse.tile as tile
from concourse.bass_utils import run_bass_kernel_spmd

# Old walrus encodes EVENT_SEMAPHORE_RANGE_CLEAR / drain-reset ranges of at
# most 9 semaphores; cap the ranges bass emits at tile-context exit.
_orig_ctr = bass.compact_to_ranges


def _capped_ctr(vals):
    out = []
    for r in _orig_ctr(vals):
        vs = list(r)
        for i in range(0, len(vs), 9):
            chunk = vs[i : i + 9]
            out.append(range(chunk[0], chunk[-1] + 1))
    return out


bass.compact_to_ranges = _capped_ctr

F32 = mybir.dt.float32
F16 = mybir.dt.float16
I32 = mybir.dt.int32
AF = mybir.ActivationFunctionType
OP = mybir.AluOpType
AX = mybir.AxisListType

KSIM = bool(int(os.environ.get("KSIM", "0")))
B, C, S = 8, 128, 16
L = S * S * S            # 4096
PX = S + 2               # 18
PL = PX * PX * PX        # 5832
NBLK = 10
NMOD = 4
NQ = 4                   # quarters per volume
QL = L // NQ             # 1024 tokens per quarter
EPS = 1e-6

# Depthwise tap split: PE diag-matmuls vs DVE/Pool fused mul-add chain.
# VEC_TAPS entries are (tap, engine) with engine "D" (DVE) or "P" (Pool).
VEC_TAPS = ((4, "D"), (13, "D"), (22, "D"), (10, "D"), (16, "D"))
PE_TAPS = tuple(t for t in range(27)
                if t not in tuple(v[0] for v in VEC_TAPS))
NPE = len(PE_TAPS)


def _mm(nc, out, lhsT, rhs, start=True, stop=True):
    nc.tensor.matmul(out, lhsT, rhs, start=start, stop=stop)


def split_wide_waits(nc, max_waits=1):
    """walrus in this container supports one sync-wait per instruction;
    move extras onto preceding no-ops on the same engine."""
    for f in nc.m.functions:
        for blk in f.blocks:
            new_insts = []
            for ins in blk.instructions:
                si = ins.sync_info
                if si is not None and si.on_wait and len(si.on_wait) > max_waits:
                    waits = list(si.on_wait)
                    k = 0
                    while len(waits) > max_waits:
                        chunk, waits = waits[:max_waits], waits[max_waits:]
                        nop = mybir.InstNoOp(
                            name=f"{ins.name}-ws{k}", ins=[], outs=[]
                        )
                        nop.engine = ins.engine
                        nop.sync_info = bass_rust.SyncInfo(
                            on_wait=chunk, on_update=[]
                        )
                        new_insts.append(nop)
                        k += 1
                    ins.sync_info = bass_rust.SyncInfo(
                        on_wait=waits, on_update=list(si.on_update or [])
                    )
                new_insts.append(ins)
            blk.instructions = new_insts


G = 32          # front guard columns in the padded volume tile


def _win(pad, tap, q):
    """Window AP into padded volume for depthwise tap, quarter q."""
    kd, r = divmod(tap, 9)
    kh, kw = divmod(r, 3)
    v = pad[:, G : G + PL].rearrange("c (x y z) -> c x y z",
                                     x=PX, y=PX, z=PX)
    return v[:, kd + 4 * q : kd + 4 * q + 4, kh : kh + 16, kw : kw + 16]


def _interior(pad, q):
    v = pad[:, G : G + PL].rearrange("c (x y z) -> c x y z",
                                     x=PX, y=PX, z=PX)
    return v[:, 1 + 4 * q : 5 + 4 * q, 1:17, 1:17]


def build_module():
    nc = bass.Bass("TRN2", target_bir_lowering=False, debug=False)

    vols_d = nc.dram_tensor("vols", [5, C, L], F16, kind="ExternalInput")
    w1t_d = nc.dram_tensor("w1t", [C, NBLK * C], F16, kind="ExternalInput")
    w2t_d = nc.dram_tensor("w2t", [C, NBLK * C], F16, kind="ExternalInput")
    bcol_d = nc.dram_tensor("bcol", [C, NBLK * 3], F32, kind="ExternalInput")
    gcol_d = nc.dram_tensor("gcol", [C, NBLK * 3], F32, kind="ExternalInput")
    bcolb_d = nc.dram_tensor("bcolb", [C, NBLK * 3], F32,
                             kind="ExternalInput")
    dwt_d = nc.dram_tensor("dwt", [C, NBLK * 27], F32, kind="ExternalInput")
    dwdiag_d = nc.dram_tensor(
        "dwdiag", [NBLK, C, NPE * C], F16, kind="ExternalInput"
    )
    identb_d = nc.dram_tensor("identb", [C, C], F16, kind="ExternalInput")
    mask_d = nc.dram_tensor("maskrow", [1, 4 * C], F16, kind="ExternalInput")
    out_d = nc.dram_tensor("out", [C, L], F16, kind="ExternalOutput")

    with tile.TileContext(nc) as tc:
        ctx = contextlib.ExitStack()
        with ctx:
            ctx.enter_context(nc.allow_low_precision(
                reason="fp16 activations; LN stats and matmuls accumulate "
                       "in fp32 PSUM"))
            csts = ctx.enter_context(tc.tile_pool(name="csts", bufs=1))
            volp = ctx.enter_context(tc.tile_pool(name="volp", bufs=2))
            scr = ctx.enter_context(tc.tile_pool(name="scr", bufs=1))
            obfp = ctx.enter_context(tc.tile_pool(name="obfp", bufs=1))
            accp = ctx.enter_context(tc.tile_pool(name="accp", bufs=2))
            padp = ctx.enter_context(tc.tile_pool(name="padp", bufs=1))
            diagp = ctx.enter_context(tc.tile_pool(name="diagp", bufs=2))
            smal = ctx.enter_context(tc.tile_pool(name="smal", bufs=1))
            psum = ctx.enter_context(
                tc.tile_pool(name="psum", bufs=1, space="PSUM")
            )

            # ---- persistent constants ----
            w1t = csts.tile([C, NBLK * C], F16)
            w2t = csts.tile([C, NBLK * C], F16)
            bcol = csts.tile([C, NBLK * 3], F32)
            bcolb = csts.tile([C, NBLK * 3], F32)
            gcol = csts.tile([C, NBLK * 3], F32)
            dwt = csts.tile([C, NBLK * 27], F32)
            identb = csts.tile([C, C], F16)
            oos = csts.tile([C, 1], F16)
            onesr = csts.tile([33, C], F16)
            urow0 = csts.tile([33, L], F16)  # p0=u, p32=vu (per-token rows)
            urow1 = csts.tile([33, L], F16)
            urows = [urow0, urow1]
            maskr = csts.tile([1, 4 * C], F16)
            qT = csts.tile([C, 32 * C], F16)
            lgacc = csts.tile([C, 4 * C], F32)
            av = csts.tile([C, L], F16)
            attnb = csts.tile([C, 4 * C], F16)
            attnT = csts.tile([C, 4 * C], F16)

            nc.sync.dma_start(out=w1t[:], in_=w1t_d.ap())
            nc.sync.dma_start(out=w2t[:], in_=w2t_d.ap())
            nc.sync.dma_start(out=bcol[:], in_=bcol_d.ap())
            nc.sync.dma_start(out=bcolb[:], in_=bcolb_d.ap())
            nc.sync.dma_start(out=gcol[:], in_=gcol_d.ap())
            nc.sync.dma_start(out=dwt[:], in_=dwt_d.ap())
            nc.sync.dma_start(out=identb[:], in_=identb_d.ap())
            nc.sync.dma_start(out=maskr[:], in_=mask_d.ap())
            nc.vector.memset(oos[:], 1.0 / 128.0)
            nc.vector.memset(onesr[:], 1.0)

            # two persistent zero-padded dw input volumes (ping-pong)
            pads = []
            for i in range(2):
                p = padp.tile([C, PL + 64], F16, tag=f"pad{i}")
                nc.vector.memset(p[:], 0.0)
                pads.append(p)

            def newton_rsqrt(par, y, v, hs):
                """y = 1/sqrt(v), fp32 [C,32] tiles, all on DVE."""
                ta = smal.tile([32, 128], F32, tag=f"nta{par}")
                nc.vector.tensor_scalar(hs[:], v[:], -0.5, None, OP.mult)
                yi = y[:].bitcast(I32)
                nc.vector.tensor_scalar(
                    yi, v[:].bitcast(I32), 1, None, OP.logical_shift_right
                )
                nc.vector.tensor_scalar(yi, yi, -1, None, OP.bitwise_xor)
                nc.vector.tensor_scalar(yi, yi, 0x5F3759E0, None, OP.add)
                for _ in range(2):
                    nc.vector.tensor_mul(ta[:], y[:], y[:])
                    nc.vector.tensor_mul(ta[:], ta[:], hs[:])
                    nc.vector.tensor_scalar(ta[:], ta[:], 1.5, None, OP.add)
                    nc.vector.tensor_mul(y[:], y[:], ta[:])

            def ln_gen(par, x_sb, blki, lnj, dst_of, gelu):
                """LN over channels. x_sb [C,L] fp16, bias already folded
                in. dst_of(q) -> output AP for quarter q."""
                g_ap = gcol[:, blki * 3 + lnj : blki * 3 + lnj + 1]
                be_ap = bcolb[:, blki * 3 + lnj : blki * 3 + lnj + 1]

                stats = smal.tile([32, 256], F32, tag=f"stats{par}")
                for hv in range(2):
                    stq = scr.tile([33, 2 * QL], F32, tag="stq", bufs=2)
                    for qq in range(2):
                        q = 2 * hv + qq
                        qs = slice(q * QL, (q + 1) * QL)
                        sq = scr.tile([C, QL], F16, tag=f"t{par}", bufs=2)
                        nc.scalar.activation(sq[:], x_sb[:, qs], AF.Square)
                        st = psum.tile([33, QL], F32, tag=f"s{par}")
                        if KSIM:
                            nc.vector.memset(st[:], 0.0)
                        for h in range(2):
                            hs = slice(q * QL + h * 512,
                                       q * QL + (h + 1) * 512)
                            ho = slice(h * 512, (h + 1) * 512)
                            _mm(nc, st[0:1, ho], oos[:], x_sb[:, hs])
                            _mm(nc, st[32:33, ho], oos[:],
                                sq[:, h * 512 : (h + 1) * 512])
                        qo = slice(qq * QL, (qq + 1) * QL)
                        nc.scalar.copy(stq[:, qo], st[:])
                        yield
                    js = slice(16 * hv, 16 * hv + 16)
                    nc.sync.dma_start(out=stats[js, 0:128], in_=stq[0:1, :])
                    nc.sync.dma_start(out=stats[js, 128:256],
                                      in_=stq[32:33, :])
                yield

                mean = stats[:, 0:128]
                var = smal.tile([32, 128], F32, tag=f"f0{par}")
                hs = smal.tile([32, 128], F32, tag=f"f1{par}")
                y = smal.tile([32, 128], F32, tag=f"f2{par}")
                u = smal.tile([32, 128], F16, tag=f"f3{par}")
                vun = smal.tile([32, 128], F16, tag=f"f4{par}")
                nc.vector.tensor_mul(var[:], mean, mean)
                nc.vector.scalar_tensor_tensor(
                    var[:], stats[:, 128:256], EPS, var[:], OP.add,
                    OP.subtract
                )
                newton_rsqrt(par, y, var, hs)
                nc.vector.tensor_copy(u[:], y[:])
                nc.vector.scalar_tensor_tensor(
                    vun[:], mean, -1.0, y[:], OP.mult, OP.mult
                )
                urow = urows[par]
                nc.sync.dma_start(out=urow[0:1, :], in_=u[:])
                nc.sync.dma_start(out=urow[32:33, :], in_=vun[:])
                yield

                for q in range(NQ):
                    qs = slice(q * QL, (q + 1) * QL)
                    ug = psum.tile([C, QL], F32, tag=f"x{par}")
                    vg = psum.tile([C, QL], F32, tag=f"s{par}")
                    for h in range(2):
                        hs2 = slice(q * QL + h * 512,
                                    q * QL + (h + 1) * 512)
                        ho = slice(h * 512, (h + 1) * 512)
                        _mm(nc, ug[:, ho], onesr[0:1, :], urow[0:1, hs2])
                        _mm(nc, vg[:, ho], onesr[32:33, :],
                            urow[32:33, hs2])
                    pre = scr.tile([C, QL], F16, tag=f"t{par}", bufs=2)
                    nc.vector.tensor_tensor(
                        pre[:], x_sb[:, qs], ug[:], OP.mult
                    )
                    nc.vector.tensor_tensor(
                        pre[:], pre[:], vg[:], OP.add
                    )
                    nc.scalar.activation(
                        dst_of(q), pre[:], AF.Gelu if gelu else AF.Identity,
                        bias=be_ap, scale=g_ap
                    )
                    yield

            def pw_gen(par, w_ap, rhs_sb, dst, b_ap):
                for q in range(NQ):
                    xq = psum.tile([C, QL], F32, tag=f"x{par}")
                    for h in range(2):
                        hs = slice(q * QL + h * 512, q * QL + (h + 1) * 512)
                        _mm(nc, xq[:, h * 512 : (h + 1) * 512], w_ap,
                            rhs_sb[:, hs])
                    nc.scalar.activation(
                        dst[:, q * QL : (q + 1) * QL], xq[:], AF.Identity,
                        bias=b_ap, scale=1.0
                    )
                    yield

            def block_gen(blki, vol_idx, dst_of):
                """Full DepthWiseConvBlock as a generator."""
                par = blki % 2
                pad = pads[par]
                vol = volp.tile([C, L], F16, tag="vol")
                nc.sync.dma_start(out=vol[:], in_=vols_d.ap()[vol_idx, :, :])
                diag = diagp.tile([C, NPE * C], F16)
                nc.sync.dma_start(out=diag[:], in_=dwdiag_d.ap()[blki, :, :])
                yield

                w1 = w1t[:, blki * C : (blki + 1) * C]
                w2 = w2t[:, blki * C : (blki + 1) * C]
                b1 = bcol[:, blki * 3 + 0 : blki * 3 + 1]
                b2 = bcol[:, blki * 3 + 1 : blki * 3 + 2]
                b3 = bcol[:, blki * 3 + 2 : blki * 3 + 3]
                x1 = scr.tile([C, L], F16, tag=f"x{par}", bufs=2)
                yield from pw_gen(par, w1, vol, x1, b1)
                yield from ln_gen(par, x1, blki, 0,
                                  lambda q: _interior(pad, q), gelu=True)

                # depthwise: per-quarter PE diag-matmul chain + DVE/Pool
                # fused mul-add chain into a dense accumulator
                x2 = scr.tile([C, L], F16, tag=f"x{par}", bufs=2)
                for q in range(NQ):
                    # 4 padded x-slabs of quarter q (incl. y/z borders)
                    bq = G + (1 + 4 * q) * 324
                    acc = accp.tile([C, 4 * 324], F16, tag=f"ac{par}",
                                    bufs=2)
                    for ti, (t, eng) in enumerate(VEC_TAPS):
                        e = nc.vector if eng == "D" else nc.gpsimd
                        kd, r = divmod(t, 9)
                        kh, kw = divmod(r, 3)
                        dlt = (kd - 1) * 324 + (kh - 1) * 18 + (kw - 1)
                        wcol = dwt[:, blki * 27 + t : blki * 27 + t + 1]
                        srcs = pad[:, bq + dlt : bq + dlt + 4 * 324]
                        if ti == 0:
                            e.tensor_scalar(acc[:], srcs, wcol, None,
                                            OP.mult)
                        else:
                            e.scalar_tensor_tensor(
                                acc[:], srcs, wcol, acc[:], OP.mult, OP.add)
                    accv = acc.rearrange("c (a y z) -> c a y z", a=4, y=18)
                    yield
                    dq = psum.tile([C, QL], F32, tag=f"x{par}")
                    for h in range(2):
                        ho = slice(h * 512, (h + 1) * 512)
                        for ti, t in enumerate(PE_TAPS):
                            w = _win(pad, t, q)
                            wh = w[:, 2 * h : 2 * h + 2, :, :]
                            _mm(nc, dq[:, ho],
                                diag[:, ti * C : (ti + 1) * C], wh,
                                start=(ti == 0), stop=False)
                        _mm(nc, dq[:, ho], identb[:],
                            accv[:, 2 * h : 2 * h + 2, 1:17, 1:17],
                            start=False, stop=True)
                    nc.scalar.activation(
                        x2[:, q * QL : (q + 1) * QL], dq[:], AF.Identity,
                        bias=b2, scale=1.0
                    )
                    yield
                x2g = scr.tile([C, L], F16, tag=f"x{par}", bufs=2)
                yield from ln_gen(par, x2, blki, 1,
                                  lambda q: x2g[:, q * QL : (q + 1) * QL],
                                  gelu=True)

                x3 = scr.tile([C, L], F16, tag=f"x{par}", bufs=2)
                yield from pw_gen(par, w2, x2g, x3, b3)
                yield from ln_gen(par, x3, blki, 2, dst_of, gelu=False)

            def transpose_gen(par, src_bf, dst_ap_of, nj=8):
                """dst_ap_of(j) -> [C, 4, C]-shaped dest AP for l-tiles
                4j..4j+3."""
                for j in range(nj):
                    tp = psum.tile([C, 4 * C], F16, tag=f"x{par}")
                    for t in range(4):
                        li = 4 * j + t
                        nc.tensor.transpose(
                            tp[:, t * C : (t + 1) * C],
                            src_bf[:, li * C : (li + 1) * C], identb[:])
                    nc.scalar.copy(
                        dst_ap_of(j),
                        tp.rearrange("c (a b) -> c a b", a=4))
                    if j % 2 == 1:
                        yield

            # ================= pipelined main program =================
            qTv = qT.rearrange("c (a b) -> c a b", b=C)

            def qblock_gen():
                qbf = obfp.tile([C, L], F16, tag="obf0", bufs=1)
                yield from block_gen(
                    0, 0, lambda q: qbf[:, q * QL : (q + 1) * QL])
                yield from transpose_gen(
                    0, qbf, lambda j: qTv[:, 4 * j : 4 * j + 4, :])

            def kblock_gen(m):
                par = (1 + m) % 2
                kbf = obfp.tile([C, L], F16, tag=f"obf{par}", bufs=1)
                yield from block_gen(
                    1 + m, 1 + m, lambda q: kbf[:, q * QL : (q + 1) * QL])
                lgm = psum.tile([C, C], F32, tag=f"s{par}")
                for ch in range(2):
                    ktmp = scr.tile([C, 16 * C], F16, tag="ktmp", bufs=1)
                    ktv = ktmp.rearrange("c (a b) -> c a b", b=C)
                    yield from transpose_gen(
                        par, kbf[:, ch * 16 * C : (ch + 1) * 16 * C],
                        lambda j: ktv[:, 4 * j : 4 * j + 4, :], nj=4)
                    for i in range(16):
                        gi = 16 * ch + i
                        _mm(nc, lgm[:], qT[:, gi * C : (gi + 1) * C],
                            ktmp[:, i * C : (i + 1) * C],
                            start=(gi == 0), stop=(gi == 31))
                    yield
                nc.vector.tensor_copy(lgacc[:, m * C : (m + 1) * C], lgm[:])
                yield

            def softmax_gen():
                mk = psum.tile([C, 4 * C], F32, tag="s1")
                _mm(nc, mk[:], onesr[0:1, :], maskr[:])
                nc.vector.tensor_scalar_mul(lgacc[:], lgacc[:],
                                            float(L) ** -0.5)
                nc.vector.tensor_add(lgacc[:], lgacc[:], mk[:])
                mx = smal.tile([C, 1], F32, tag="g0")
                nc.vector.tensor_reduce(mx[:], lgacc[:], AX.X, OP.max)
                nc.vector.tensor_scalar_sub(lgacc[:], lgacc[:], mx[:])
                nc.scalar.activation(lgacc[:], lgacc[:], AF.Exp)
                sm = smal.tile([C, 1], F32, tag="g1")
                nc.vector.tensor_reduce(sm[:], lgacc[:], AX.X, OP.add)
                nc.vector.reciprocal(sm[:], sm[:])
                nc.vector.tensor_scalar_mul(attnb[:], lgacc[:], sm[:])
                yield
                tp = psum.tile([C, 4 * C], F16, tag="s1")
                for t in range(4):
                    nc.tensor.transpose(
                        tp[:, t * C : (t + 1) * C],
                        attnb[:, t * C : (t + 1) * C], identb[:])
                nc.vector.tensor_copy(attnT[:], tp[:])
                yield

            def vblock_gen(m):
                par = (1 + m) % 2
                vout = obfp.tile([C, L], F16, tag=f"obf{par}", bufs=1)
                yield from block_gen(
                    5 + m, 1 + m, lambda q: vout[:, q * QL : (q + 1) * QL])
                for q in range(NQ):
                    aq = psum.tile([C, QL], F32, tag=f"x{par}")
                    for h in range(2):
                        hs = slice(q * QL + h * 512, q * QL + (h + 1) * 512)
                        _mm(nc, aq[:, h * 512 : (h + 1) * 512],
                            attnT[:, m * C : (m + 1) * C], vout[:, hs])
                    avq = av[:, q * QL : (q + 1) * QL]
                    if m == 0:
                        nc.vector.tensor_copy(avq, aq[:])
                    else:
                        nc.vector.tensor_add(avq, avq, aq[:])
                    yield

            # block 9 reads av instead of a DRAM volume
            def block_gen9(blki, vol_idx, dst_of):
                par = blki % 2
                pad = pads[par]
                diag = diagp.tile([C, NPE * C], F16)
                nc.sync.dma_start(out=diag[:], in_=dwdiag_d.ap()[blki, :, :])
                yield

                w1 = w1t[:, blki * C : (blki + 1) * C]
                w2 = w2t[:, blki * C : (blki + 1) * C]
                b1 = bcol[:, blki * 3 + 0 : blki * 3 + 1]
                b2 = bcol[:, blki * 3 + 1 : blki * 3 + 2]
                b3 = bcol[:, blki * 3 + 2 : blki * 3 + 3]
                x1 = scr.tile([C, L], F16, tag=f"x{par}", bufs=2)
                yield from pw_gen(par, w1, av, x1, b1)
                yield from ln_gen(par, x1, blki, 0,
                                  lambda q: _interior(pad, q), gelu=True)
                x2 = scr.tile([C, L], F16, tag=f"x{par}", bufs=2)
                for q in range(NQ):
                    # 4 padded x-slabs of quarter q (incl. y/z borders)
                    bq = G + (1 + 4 * q) * 324
                    acc = accp.tile([C, 4 * 324], F16, tag=f"ac{par}",
                                    bufs=2)
                    for ti, (t, eng) in enumerate(VEC_TAPS):
                        e = nc.vector if eng == "D" else nc.gpsimd
                        kd, r = divmod(t, 9)
                        kh, kw = divmod(r, 3)
                        dlt = (kd - 1) * 324 + (kh - 1) * 18 + (kw - 1)
                        wcol = dwt[:, blki * 27 + t : blki * 27 + t + 1]
                        srcs = pad[:, bq + dlt : bq + dlt + 4 * 324]
                        if ti == 0:
                            e.tensor_scalar(acc[:], srcs, wcol, None,
                                            OP.mult)
                        else:
                            e.scalar_tensor_tensor(
                                acc[:], srcs, wcol, acc[:], OP.mult, OP.add)
                    accv = acc.rearrange("c (a y z) -> c a y z", a=4, y=18)
                    yield
                    dq = psum.tile([C, QL], F32, tag=f"x{par}")
                    for h in range(2):
                        ho = slice(h * 512, (h + 1) * 512)
                        for ti, t in enumerate(PE_TAPS):
                            w = _win(pad, t, q)
                            wh = w[:, 2 * h : 2 * h + 2, :, :]
                            _mm(nc, dq[:, ho],
                                diag[:, ti * C : (ti + 1) * C], wh,
                                start=(ti == 0), stop=False)
                        _mm(nc, dq[:, ho], identb[:],
                            accv[:, 2 * h : 2 * h + 2, 1:17, 1:17],
                            start=False, stop=True)
                    nc.scalar.activation(
                        x2[:, q * QL : (q + 1) * QL], dq[:], AF.Identity,
                        bias=b2, scale=1.0
                    )
                    yield
                x2g = scr.tile([C, L], F16, tag=f"x{par}", bufs=2)
                yield from ln_gen(par, x2, blki, 1,
                                  lambda q: x2g[:, q * QL : (q + 1) * QL],
                                  gelu=True)
                x3 = scr.tile([C, L], F16, tag=f"x{par}", bufs=2)
                yield from pw_gen(par, w2, x2g, x3, b3)
                yield from ln_gen(par, x3, blki, 2, dst_of, gelu=False)

            def outblock_gen2():
                fout = obfp.tile([C, L], F16, tag="obf1", bufs=1)
                yield from block_gen9(
                    9, None, lambda q: fout[:, q * QL : (q + 1) * QL])
                nc.sync.dma_start(out=out_d.ap(), in_=fout[:])
                yield

            def chain(*gens):
                for g in gens:
                    yield from g

            kdbg = int(os.environ.get("KDBG", "0"))
            if kdbg == 1:
                def dbg_gen():
                    qbf = obfp.tile([C, L], F16, tag="obf0", bufs=1)
                    yield from block_gen(
                        0, 0, lambda q: qbf[:, q * QL : (q + 1) * QL])
                    nc.sync.dma_start(out=out_d.ap(), in_=qbf[:])
                    yield
                queue = [dbg_gen()]
            elif kdbg == 2:
                def dbg2_gen():
                    yield from softmax_gen()
                    nc.sync.dma_start(out=out_d.ap()[:, 0:4 * C],
                                      in_=attnb[:])
                    yield
                queue = [qblock_gen()]
                for m in range(NMOD - 1):
                    queue.append(kblock_gen(m))
                queue.append(chain(kblock_gen(NMOD - 1), dbg2_gen()))
            elif kdbg == 4:
                def dbg4_gen():
                    vout = obfp.tile([C, L], F16, tag="obf1", bufs=1)
                    yield from block_gen(
                        5, 1, lambda q: vout[:, q * QL : (q + 1) * QL])
                    nc.sync.dma_start(out=out_d.ap(), in_=vout[:])
                    yield
                queue = [qblock_gen()]
                for m in range(NMOD - 1):
                    queue.append(kblock_gen(m))
                queue.append(chain(kblock_gen(NMOD - 1), softmax_gen()))
                queue.append(dbg4_gen())
            elif kdbg == 3:
                def dbg3_gen():
                    nc.sync.dma_start(out=out_d.ap(), in_=av[:])
                    yield
                queue = [qblock_gen()]
                for m in range(NMOD - 1):
                    queue.append(kblock_gen(m))
                queue.append(chain(kblock_gen(NMOD - 1), softmax_gen()))
                for m in range(NMOD - 1):
                    queue.append(vblock_gen(m))
                queue.append(chain(vblock_gen(NMOD - 1), dbg3_gen()))
            else:
                queue = [qblock_gen()]
                for m in range(NMOD - 1):
                    queue.append(kblock_gen(m))
                queue.append(chain(kblock_gen(NMOD - 1), softmax_gen()))
                for m in range(NMOD - 1):
                    queue.append(vblock_gen(m))
                queue.append(chain(vblock_gen(NMOD - 1), outblock_gen2()))

            STAG = 24
            active = []   # [generator, yields_taken]
            while active or queue:
                if queue and (not active or
                              (len(active) < 2 and active[-1][1] >= STAG)):
                    active.append([queue.pop(0), 0])
                for ent in list(active):
                    try:
                        next(ent[0])
                        ent[1] += 1
                    except StopIteration:
                        active.remove(ent)

    split_wide_waits(nc)
    return nc


_CACHED = {}
_RUN_KWARGS = {}
_LAST_RESULT = None


def _build():
    if "nc" not in _CACHED:
        _CACHED["nc"] = build_module()
    return _CACHED["nc"]


def _f16(x):
    return np.asarray(x, dtype=np.float16)


def prepare(**inputs):
    """Build (nc, in_maps) without running. Shared by kernel() and sim.py."""
    query = np.asarray(inputs["query"], np.float32)
    mods = [np.asarray(inputs[k], np.float32)
            for k in ("flair", "t1ce", "t1", "t2")]
    mask = np.asarray(inputs["mask"])
    p = {k: np.asarray(inputs[k], np.float32) for k in
         ("pw1_w", "pw1_b", "ln1_g", "ln1_b", "dw_w", "dw_b",
          "ln2_g", "ln2_b", "pw2_w", "pw2_b", "ln3_g", "ln3_b")}

    w1t = _f16(p["pw1_w"].transpose(2, 0, 1).reshape(C, NBLK * C))
    w2t = _f16(p["pw2_w"].transpose(2, 0, 1).reshape(C, NBLK * C))
    bst = np.stack([p["pw1_b"], p["dw_b"], p["pw2_b"]], axis=1)  # [10,3,C]
    gst = np.stack([p["ln1_g"], p["ln2_g"], p["ln3_g"]], axis=1)
    best = np.stack([p["ln1_b"], p["ln2_b"], p["ln3_b"]], axis=1)
    bcol = np.ascontiguousarray(bst.transpose(2, 0, 1).reshape(C, NBLK * 3))
    gcol = np.ascontiguousarray(gst.transpose(2, 0, 1).reshape(C, NBLK * 3))
    bcolb = np.ascontiguousarray(
        best.transpose(2, 0, 1).reshape(C, NBLK * 3))
    dwt = p["dw_w"].reshape(NBLK, C, 27)
    dwtc = np.ascontiguousarray(
        dwt.transpose(1, 0, 2).reshape(C, NBLK * 27))
    dwdiag = np.zeros((NBLK, C, NPE, C), np.float32)
    ii = np.arange(C)
    for k, t in enumerate(PE_TAPS):
        dwdiag[:, ii, k, ii] = dwt[:, :, t]
    dwdiag = _f16(dwdiag.reshape(NBLK, C, NPE * C))
    identb = _f16(np.eye(C, dtype=np.float32))

    nc = _build()

    shared = dict(w1t=w1t, w2t=w2t, bcol=bcol, bcolb=bcolb, gcol=gcol,
                  dwt=dwtc, dwdiag=dwdiag, identb=identb)
    in_maps = []
    for b in range(B):
        vols = _f16(np.stack(
            [query[b].reshape(C, L)] + [m[b].reshape(C, L) for m in mods]))
        mrow = _f16(np.where(np.repeat(mask[b] > 0, C), 0.0,
                             -60000.0)[None, :])
        in_maps.append(dict(vols=vols, maskrow=mrow, **shared))
    return nc, in_maps


def postprocess_one(out):
    return np.asarray(out, np.float32).reshape(C, S, S, S)


def kernel(**inputs):
    global _LAST_RESULT
    nc, in_maps = prepare(**inputs)
    res = run_bass_kernel_spmd(nc, in_maps, core_ids=list(range(B)),
                               **_RUN_KWARGS)
    _LAST_RESULT = res
    query = np.asarray(inputs["query"], np.float32)
    x = np.stack([postprocess_one(res.results[b]["out"]) for b in range(B)])
    return (query + x).astype(np.float32)
